# revision 1
# baseline (speedup 1.0000x reference)
"""CGCNN conv kernel for 8 TRN2 NeuronCores (Bass/Tile).

Strategy (edge-parallel, dst-sharded):
  z @ W = nf[src] @ W[0:64] + nf[dst] @ W[64:128] + ef @ W[128:160]
  - Host precomputes P_src = nf @ [Wi[:64]|Wu[:64]]  (bf16 [N,128], 256B rows)
                    P_dst = nf @ [Wi[64:128]|Wu[64:128]]
  - Edges sorted by (dst//R, src//CH, src): core c owns dst range
    [c*R,(c+1)*R) so the segment-sum is core-local (no [N,F] all-reduce);
    within a core edges are grouped into src-chunks of CH=25000 so gather
    indices fit int16.
  - Pass 1: transposed dma_gather of P rows -> feat-major [128,T] tiles;
    PE adds the edge-feat matmul; DVE ttr assembles x (+sum); ACT
    square-accum (+sumsq); x stored to DRAM bf16.
  - AllReduce [128,2] edge-BN stats.
  - Pass 2: reload x; gate = Sigmoid(s*x+b); softplus via Ln(1+Exp(.))
    (no softplus table on TRN2); msg transposed to row-major on PE;
    dma_scatter_add into per-core agg [R_pad, 64].
  - Phase 3: node-BN stats AllReduce [64,2]; out = softplus(nf + bn(agg))
    computed feat-major; host transposes back.
"""

import math
import sys

import numpy as np

for _p in ("/opt/trn_rl_repo", "/root/.axon_site/_ro/trn_rl_repo"):
    if _p not in sys.path:
        sys.path.append(_p)

import ml_dtypes
from concourse import bacc, bass, mybir
from concourse import tile as ctile
from concourse.bass_utils import run_bass_kernel_spmd
from concourse.masks import make_identity

P = 128
F = 64  # node feature dim; 2F == P
EPS = 1e-5
BF16 = ml_dtypes.bfloat16

Alu = mybir.AluOpType
Act = mybir.ActivationFunctionType
dt = mybir.dt


def _cfg(N, E, FE, T=2048, sub=512, g_batch=6, ncores=8):
    R = N // ncores
    assert R * ncores == N
    nchunk = max(1, math.ceil(N / 25000))
    CH = math.ceil(N / nchunk)
    assert CH + 1 <= 32768 and R + 1 <= 32768
    r_pad = math.ceil((R + 1) / P) * P
    return dict(
        N=N, E=E, FE=FE, T=T, SUB=sub, G=g_batch, NC=ncores,
        R=R, NCHUNK=nchunk, CH=CH, R_PAD=r_pad,
    )


def build_graph(cfg, debug=False):
    NC, T, SUB, FE = cfg["NC"], cfg["T"], cfg["SUB"], cfg["FE"]
    CH, NCHUNK, R_PAD = cfg["CH"], cfg["NCHUNK"], cfg["R_PAD"]
    TPC, ETOT = cfg["TPC"], cfg["ETOT"]
    SEGS = list(cfg["SEGS"])
    nseg = len(SEGS)
    NTILES = NCHUNK * TPC
    NBLK = NTILES // 2
    NGRP = R_PAD // P
    nsub = T // SUB
    inv_e = 1.0 / float(cfg["E"])
    inv_n = 1.0 / float(cfg["N"])

    nc = bacc.Bacc("TRN2", target_bir_lowering=False, debug=False,
                   num_devices=NC)

    psrc = [nc.dram_tensor(f"psrc{c}", [CH + 1, P], dt.bfloat16,
                           kind="ExternalInput") for c in range(NCHUNK)]
    pdst = nc.dram_tensor("pdst", [R_PAD, P], dt.bfloat16, kind="ExternalInput")
    eft = nc.dram_tensor("eft", [FE, ETOT], dt.bfloat16, kind="ExternalInput")
    srcidx = nc.dram_tensor("srcidx", [P, ETOT // 16], dt.int16,
                            kind="ExternalInput")
    dstidx = nc.dram_tensor("dstidx", [P, ETOT // 16], dt.int16,
                            kind="ExternalInput")
    nft = nc.dram_tensor("nft", [F, R_PAD], dt.float32, kind="ExternalInput")
    w3 = nc.dram_tensor("w3", [FE, P], dt.bfloat16, kind="ExternalInput")
    gvec = nc.dram_tensor("gvec", [P, 1], dt.float32, kind="ExternalInput")
    bvec = nc.dram_tensor("bvec", [P, 1], dt.float32, kind="ExternalInput")
    gbn = nc.dram_tensor("gbn", [F, 1], dt.float32, kind="ExternalInput")
    bbn = nc.dram_tensor("bbn", [F, 1], dt.float32, kind="ExternalInput")
    outT = nc.dram_tensor("outT", [F, R_PAD], dt.float32, kind="ExternalOutput")

    xint = nc.dram_tensor("xint", [NBLK, P, T], dt.bfloat16, kind="Internal")
    xupd = nc.dram_tensor("xupd", [NBLK, P, T], dt.bfloat16, kind="Internal")
    aggd = [nc.dram_tensor(f"aggd{r}", [NGRP, P, F], dt.float32,
                           kind="Internal") for r in range(nseg + 1)]
    cc1i = nc.dram_tensor("cc1i", [P, 2], dt.float32, kind="Internal")
    cc1o = nc.dram_tensor("cc1o", [P, 2], dt.float32, kind="Internal",
                          addr_space="Shared")
    cc2i = nc.dram_tensor("cc2i", [F, 2], dt.float32, kind="Internal")
    cc2o = nc.dram_tensor("cc2o", [F, 2], dt.float32, kind="Internal",
                          addr_space="Shared")

    rg = [list(range(NC))]
    if debug:
        dbg_xint = nc.dram_tensor("dbg_xint", [NBLK, P, T], dt.bfloat16,
                                  kind="ExternalOutput")
        dbg_agg = nc.dram_tensor("dbg_agg", [NGRP, P, F], dt.float32,
                                 kind="ExternalOutput")
        dbg_st = nc.dram_tensor("dbg_st", [P, 12], dt.float32,
                                kind="ExternalOutput")

    with ctile.TileContext(nc) as tc:
        with tc.tile_pool(name="const", bufs=1) as cp:
            w3_sb = cp.tile([FE, P], dt.bfloat16)
            nc.sync.dma_start(w3_sb[:], w3.ap())
            identb = cp.tile([P, P], dt.bfloat16)
            make_identity(nc, identb[:])
            identf = cp.tile([F, F], dt.float32)
            make_identity(nc, identf[:])
            identf128 = cp.tile([P, P], dt.float32)
            make_identity(nc, identf128[:])
            gv = cp.tile([P, 1], dt.float32)
            nc.sync.dma_start(gv[:], gvec.ap())
            bv = cp.tile([P, 1], dt.float32)
            nc.sync.dma_start(bv[:], bvec.ap())
            gbn_sb = cp.tile([F, 1], dt.float32)
            nc.sync.dma_start(gbn_sb[:], gbn.ap())
            bbn_sb = cp.tile([F, 1], dt.float32)
            nc.sync.dma_start(bbn_sb[:], bbn.ap())

            sumc = cp.tile([P, NTILES * nsub], dt.float32)
            sqc = cp.tile([P, NTILES * nsub], dt.float32)

            # zero-fill agg accumulator
            zb = cp.tile([P, SUB], dt.float32)
            nc.vector.memset(zb[:], 0.0)
            gper = SUB // F  # groups of [P,F] per zero DMA
            for r in range(nseg + 1):
                for g0 in range(0, NGRP, gper):
                    ng = min(gper, NGRP - g0)
                    nc.sync.dma_start(aggd[r].ap()[g0:g0 + ng, :, :],
                                      zb[:, :ng * F])

            # ---------------- pass 1 ----------------
            with tc.tile_pool(name="p1", bufs=4) as p1, \
                 tc.tile_pool(name="p1i", bufs=6) as p1i, \
                 tc.tile_pool(name="ps1", bufs=4, space="PSUM") as ps1:
                for c in range(NCHUNK):
                    for tl in range(TPC):
                        t = c * TPC + tl
                        col0 = t * (T // 16)
                        sidx = p1i.tile([P, T // 16], dt.int16, tag="sidx")
                        nc.sync.dma_start(sidx[:],
                                          srcidx.ap()[:, col0:col0 + T // 16])
                        didx = p1i.tile([P, T // 16], dt.int16, tag="didx")
                        nc.sync.dma_start(didx[:],
                                          dstidx.ap()[:, col0:col0 + T // 16])
                        # transposed dma_gather crashes the device above 512
                        # indices per call -- split into 512-index sub-calls
                        GQ = 512
                        srcg = p1.tile([P, 1, T], dt.bfloat16, tag="srcg")
                        dstg = p1.tile([P, 1, T], dt.bfloat16, tag="dstg")
                        for q in range(T // GQ):
                            qs = slice(q * GQ, (q + 1) * GQ)
                            qi = slice(q * (GQ // 16), (q + 1) * (GQ // 16))
                            nc.gpsimd.dma_gather(
                                srcg[:, :, qs], psrc[c].ap(), sidx[:, qi],
                                GQ, GQ, P, transpose=True)
                            nc.gpsimd.dma_gather(
                                dstg[:, :, qs], pdst.ap(), didx[:, qi],
                                GQ, GQ, P, transpose=True)
                        eftt = p1.tile([FE, T], dt.bfloat16, tag="eftt")
                        nc.sync.dma_start(eftt[:], eft.ap()[:, t * T:(t + 1) * T])

                        x_sb = p1.tile([P, T], dt.bfloat16, tag="x")
                        sqd = p1.tile([P, SUB], dt.bfloat16, tag="sqd")
                        for s in range(nsub):
                            sl = slice(s * SUB, (s + 1) * SUB)
                            ps = ps1.tile([P, SUB], dt.float32, tag="ps")
                            nc.tensor.matmul(ps[:], w3_sb[:], eftt[:, sl],
                                             start=True, stop=False)
                            nc.tensor.matmul(ps[:], identb[:], srcg[:, 0, sl],
                                             start=False, stop=True)
                            scol = t * nsub + s
                            nc.vector.tensor_tensor(
                                x_sb[:, sl], ps[:], dstg[:, 0, sl], Alu.add)
                            nc.vector.tensor_reduce(
                                sumc[:, scol:scol + 1], x_sb[:, sl],
                                mybir.AxisListType.X, Alu.add)
                            nc.scalar.activation(
                                sqd[:], x_sb[:, sl], Act.Square,
                                accum_out=sqc[:, scol:scol + 1])
                        blk, half = t // 2, (t % 2) * F
                        nc.scalar.dma_start(xint.ap()[blk, half:half + F, :],
                                            x_sb[0:F, :])
                        nc.scalar.dma_start(xupd.ap()[blk, half:half + F, :],
                                            x_sb[F:P, :])

            # ---------------- edge-BN stats ----------------
            sums = cp.tile([P, 2], dt.float32)
            nc.vector.tensor_reduce(sums[:, 0:1], sumc[:],
                                    mybir.AxisListType.X, Alu.add)
            nc.vector.tensor_reduce(sums[:, 1:2], sqc[:],
                                    mybir.AxisListType.X, Alu.add)
            nc.sync.dma_start(cc1i.ap(), sums[:])
            nc.gpsimd.collective_compute(
                "AllReduce", Alu.add, replica_groups=rg,
                ins=[cc1i.ap().opt()], outs=[cc1o.ap().opt()])
            gstats = cp.tile([P, 2], dt.float32)
            nc.sync.dma_start(gstats[:], cc1o.ap())

            mu = cp.tile([P, 1], dt.float32)
            nc.vector.tensor_scalar(mu[:], gstats[:, 0:1], inv_e, None, Alu.mult)
            veps = cp.tile([P, 1], dt.float32)
            # E[x^2] - mu^2 + eps  ==  (sq*inv_e - mu*mu) + eps
            musq = cp.tile([P, 1], dt.float32)
            nc.vector.tensor_tensor(musq[:], mu[:], mu[:], Alu.mult)
            nc.vector.tensor_scalar(veps[:], gstats[:, 1:2], inv_e, None,
                                    Alu.mult)
            nc.vector.tensor_tensor(veps[:], veps[:], musq[:], Alu.subtract)
            nc.vector.tensor_scalar(veps[:], veps[:], EPS, None, Alu.add)
            sdv = cp.tile([P, 1], dt.float32)
            nc.scalar.sqrt(sdv[:], veps[:])
            isd = cp.tile([P, 1], dt.float32)
            nc.vector.reciprocal(isd[:], sdv[:])
            scl = cp.tile([P, 1], dt.float32)
            nc.vector.tensor_tensor(scl[:], gv[:], isd[:], Alu.mult)
            shf = cp.tile([P, 1], dt.float32)
            nc.vector.tensor_tensor(shf[:], mu[:], scl[:], Alu.mult)
            nc.vector.tensor_tensor(shf[:], bv[:], shf[:], Alu.subtract)

            # duplicate halves: sig_* from rows 0:F, exp_* from rows F:P
            sig_s = cp.tile([P, 1], dt.float32)
            sig_b = cp.tile([P, 1], dt.float32)
            exp_s = cp.tile([P, 1], dt.float32)
            exp_b = cp.tile([P, 1], dt.float32)
            nc.vector.tensor_copy(sig_s[0:F, :], scl[0:F, :])
            nc.sync.dma_start(sig_s[F:P, :], scl[0:F, :])
            nc.vector.tensor_copy(sig_b[0:F, :], shf[0:F, :])
            nc.sync.dma_start(sig_b[F:P, :], shf[0:F, :])
            nc.sync.dma_start(exp_s[0:F, :], scl[F:P, :])
            nc.vector.tensor_copy(exp_s[F:P, :], scl[F:P, :])
            nc.sync.dma_start(exp_b[0:F, :], shf[F:P, :])
            nc.vector.tensor_copy(exp_b[F:P, :], shf[F:P, :])

            if debug:
                nc.sync.dma_start(dbg_xint.ap(), xint.ap())
                dstt = cp.tile([P, 12], dt.float32)
                nc.vector.tensor_copy(dstt[:, 0:2], sums[:])
                nc.vector.tensor_copy(dstt[:, 2:4], gstats[:])
                nc.vector.tensor_copy(dstt[:, 4:5], scl[:])
                nc.vector.tensor_copy(dstt[:, 5:6], shf[:])
                nc.vector.tensor_copy(dstt[:, 6:7], sig_s[:])
                nc.vector.tensor_copy(dstt[:, 7:8], sig_b[:])
                nc.vector.tensor_copy(dstt[:, 8:9], exp_s[:])
                nc.vector.tensor_copy(dstt[:, 9:10], exp_b[:])
                nc.sync.dma_start(dbg_st.ap(), dstt[:])

            # ---------------- pass 2 ----------------
            G = cfg["G"]
            with tc.tile_pool(name="p2g", bufs=G + 2) as p2g, \
                 tc.tile_pool(name="p2", bufs=3) as p2, \
                 tc.tile_pool(name="p2i", bufs=3) as p2i, \
                 tc.tile_pool(name="ps2", bufs=4, space="PSUM") as ps2:
                for b0 in range(0, NBLK, G):
                    blks = range(b0, min(b0 + G, NBLK))
                    gates = {}
                    for b in blks:
                        xi = p2.tile([P, T], dt.bfloat16, tag="xi")
                        nc.sync.dma_start(xi[:], xint.ap()[b, :, :])
                        gate = p2g.tile([P, T], dt.bfloat16, tag="gate")
                        nc.scalar.activation(gate[:], xi[:], Act.Sigmoid,
                                             bias=sig_b[:], scale=sig_s[:])
                        gates[b] = gate
                    for b in blks:
                        xu = p2.tile([P, T], dt.bfloat16, tag="xu")
                        nc.sync.dma_start(xu[:], xupd.ap()[b, :, :])
                        u = p2.tile([P, T], dt.float32, tag="u")
                        nc.scalar.activation(u[:], xu[:], Act.Exp,
                                             bias=exp_b[:], scale=exp_s[:])
                        sp = p2.tile([P, T], dt.float32, tag="sp")
                        nc.scalar.activation(sp[:], u[:], Act.Ln, bias=1.0,
                                             scale=1.0)
                        gate = gates.pop(b)
                        msga = p2.tile([F, T], dt.float32, tag="msga")
                        nc.vector.tensor_tensor(msga[:], gate[0:F, :],
                                                sp[0:F, :], Alu.mult)
                        msgb = p2.tile([F, T], dt.float32, tag="msgb")
                        nc.vector.tensor_tensor(msgb[:], gate[F:P, :],
                                                sp[F:P, :], Alu.mult)
                        ssrc = p2.tile([P, 2 * T // P, F], dt.float32,
                                       tag="ssrc")
                        ntr = T // P  # transposes per msg half
                        per_ps = SUB // F  # transposed [P,F] blocks per psum
                        for q in range(2 * T // (P * per_ps)):
                            pst = ps2.tile([P, SUB], dt.float32, tag="pst")
                            for k in range(per_ps):
                                j = q * per_ps + k
                                src = msga if j < ntr else msgb
                                jj = j % ntr
                                nc.tensor.transpose(
                                    pst[:, k * F:(k + 1) * F],
                                    src[:, jj * P:(jj + 1) * P],
                                    identf[:])
                            nc.vector.tensor_copy(
                                ssrc[:, q * per_ps:(q + 1) * per_ps, :],
                                pst[:])
                        didx2 = p2i.tile([P, 2 * T // 16], dt.int16,
                                         tag="didx2")
                        nc.sync.dma_start(
                            didx2[:],
                            dstidx.ap()[:, b * (2 * T // 16):
                                        (b + 1) * (2 * T // 16)])
                        off = 0
                        for r, sr in enumerate(SEGS):
                            ri = (nseg if (r == 0 and b % 2) else r)
                            nc.gpsimd.dma_scatter_add(
                                aggd[ri].ap().flatten_outer_dims(),
                                ssrc[:, off // P:(off + sr) // P, :],
                                didx2[:, off // 16:(off + sr) // 16],
                                sr, sr, F)
                            off += sr

            if debug:
                nc.sync.dma_start(dbg_agg.ap(), aggd[0].ap())

            # ---------------- phase 3 (chunked over node groups) ------
            with tc.tile_pool(name="p3", bufs=1) as p3, \
                 tc.tile_pool(name="p3c", bufs=3) as p3c, \
                 tc.tile_pool(name="p3w", bufs=2) as p3w, \
                 tc.tile_pool(name="ps3", bufs=4, space="PSUM") as ps3:
                gpt = SUB // P  # groups per psum tile / chunk
                aggT = p3.tile([F, NGRP * P], dt.float32)
                for q0 in range(0, NGRP, gpt):
                    nq = min(gpt, NGRP - q0)
                    ac = p3c.tile([P, gpt, F], dt.float32, tag="ac")
                    nc.sync.dma_start(
                        ac[:, :nq, :],
                        aggd[0].ap()[q0:q0 + nq].rearrange("g p d -> p g d"))
                    for r in range(1, nseg + 1):
                        at = p3c.tile([P, gpt, F], dt.float32, tag="at")
                        nc.sync.dma_start(
                            at[:, :nq, :],
                            aggd[r].ap()[q0:q0 + nq].rearrange("g p d -> p g d"))
                        nc.vector.tensor_tensor(ac[:, :nq, :], ac[:, :nq, :],
                                                at[:, :nq, :], Alu.add)
                    pst = ps3.tile([F, SUB], dt.float32, tag="pst3")
                    for k in range(nq):
                        nc.tensor.transpose(
                            pst[:, k * P:(k + 1) * P],
                            ac[:, k, :], identf128[:])
                    nc.vector.tensor_copy(
                        aggT[:, q0 * P:(q0 + nq) * P], pst[:, :nq * P])

                Rr = cfg["R"]
                if R_PAD > Rr:
                    # zero trash-node columns so pad values stay bounded
                    nc.vector.memset(aggT[:, Rr:], 0.0)
                nchunk3 = 8
                cb = [(Rr * i) // nchunk3 for i in range(nchunk3 + 1)]
                nsum = p3.tile([F, 2 * nchunk3], dt.float32)
                for i in range(nchunk3):
                    sl = slice(cb[i], cb[i + 1])
                    nc.vector.tensor_reduce(nsum[:, 2 * i:2 * i + 1],
                                            aggT[:, sl],
                                            mybir.AxisListType.X, Alu.add)
                    sq = p3w.tile([F, (NGRP * P) // nchunk3 + P], dt.float32,
                                  tag="sq")
                    w = cb[i + 1] - cb[i]
                    nc.vector.tensor_tensor(sq[:, :w], aggT[:, sl],
                                            aggT[:, sl], Alu.mult)
                    nc.vector.tensor_reduce(nsum[:, 2 * i + 1:2 * i + 2],
                                            sq[:, :w],
                                            mybir.AxisListType.X, Alu.add)
                nsum2 = p3.tile([F, 2], dt.float32)
                nc.vector.tensor_reduce(
                    nsum2[:, 0:1],
                    nsum[:].rearrange("p (a b) -> p b a", b=2)[:, 0, :],
                    mybir.AxisListType.X, Alu.add)
                nc.vector.tensor_reduce(
                    nsum2[:, 1:2],
                    nsum[:].rearrange("p (a b) -> p b a", b=2)[:, 1, :],
                    mybir.AxisListType.X, Alu.add)
                nsum = nsum2
                nc.sync.dma_start(cc2i.ap(), nsum[:])
                nc.gpsimd.collective_compute(
                    "AllReduce", Alu.add, replica_groups=rg,
                    ins=[cc2i.ap().opt()], outs=[cc2o.ap().opt()])
                gs2 = p3.tile([F, 2], dt.float32)
                nc.sync.dma_start(gs2[:], cc2o.ap())

                mu2 = p3.tile([F, 1], dt.float32)
                nc.vector.tensor_scalar(mu2[:], gs2[:, 0:1], inv_n, None,
                                        Alu.mult)
                ve2 = p3.tile([F, 1], dt.float32)
                ms2 = p3.tile([F, 1], dt.float32)
                nc.vector.tensor_tensor(ms2[:], mu2[:], mu2[:], Alu.mult)
                nc.vector.tensor_scalar(ve2[:], gs2[:, 1:2], inv_n, None,
                                        Alu.mult)
                nc.vector.tensor_tensor(ve2[:], ve2[:], ms2[:], Alu.subtract)
                nc.vector.tensor_scalar(ve2[:], ve2[:], EPS, None, Alu.add)
                sd2 = p3.tile([F, 1], dt.float32)
                nc.scalar.sqrt(sd2[:], ve2[:])
                is2 = p3.tile([F, 1], dt.float32)
                nc.vector.reciprocal(is2[:], sd2[:])
                sc2 = p3.tile([F, 1], dt.float32)
                nc.vector.tensor_tensor(sc2[:], gbn_sb[:], is2[:], Alu.mult)
                sh2 = p3.tile([F, 1], dt.float32)
                nc.vector.tensor_tensor(sh2[:], mu2[:], sc2[:], Alu.mult)
                nc.vector.tensor_tensor(sh2[:], bbn_sb[:], sh2[:], Alu.subtract)

                cw = ((NGRP // nchunk3) + 1) * P
                for i in range(nchunk3):
                    c0 = min(NGRP * P, i * cw)
                    c1 = min(NGRP * P, (i + 1) * cw)
                    if c1 <= c0:
                        continue
                    w = c1 - c0
                    nftc = p3w.tile([F, cw], dt.float32, tag="nftc")
                    nc.sync.dma_start(nftc[:, :w], nft.ap()[:, c0:c1])
                    s1 = p3w.tile([F, cw], dt.float32, tag="s1")
                    nc.vector.tensor_scalar(s1[:, :w], aggT[:, c0:c1],
                                            sc2[:], sh2[:], Alu.mult, Alu.add)
                    nc.vector.tensor_tensor(s1[:, :w], s1[:, :w], nftc[:, :w],
                                            Alu.add)
                    u3 = p3w.tile([F, cw], dt.float32, tag="u3")
                    nc.scalar.activation(u3[:, :w], s1[:, :w], Act.Exp)
                    o3 = p3w.tile([F, cw], dt.float32, tag="o3")
                    nc.scalar.activation(o3[:, :w], u3[:, :w], Act.Ln,
                                         bias=1.0, scale=1.0)
                    nc.sync.dma_start(outT.ap()[:, c0:c1], o3[:, :w])

    nc.compile()
    return nc


_CACHE = {}


def _prep(inputs, T=2048, g_batch=6):
    nf = np.ascontiguousarray(np.asarray(inputs["node_feats"], np.float32))
    ef = np.ascontiguousarray(np.asarray(inputs["edge_feats"], np.float32))
    src = np.asarray(inputs["src"], np.int64)
    dst = np.asarray(inputs["dst"], np.int64)
    Wi = np.asarray(inputs["W_int"], np.float32)
    Wu = np.asarray(inputs["W_upd"], np.float32)
    N, Fn = nf.shape
    E, FE = ef.shape
    assert Fn == F
    cfg = _cfg(N, E, FE, T=T, g_batch=g_batch)
    NCh, CH, R, NCc = cfg["NCHUNK"], cfg["CH"], cfg["R"], cfg["NC"]

    # b_int/b_upd are dropped: a constant bias shifts mean equally and
    # cancels inside BatchNorm.
    Psrc = (nf @ np.concatenate([Wi[:F], Wu[:F]], axis=1)).astype(BF16)
    Pdst = (nf @ np.concatenate([Wi[F:2 * F], Wu[F:2 * F]], axis=1)).astype(BF16)
    W3 = np.concatenate([Wi[2 * F:], Wu[2 * F:]], axis=1).astype(BF16)

    core = dst // R
    chunk = src // CH
    key = core * NCh + chunk
    order = np.lexsort((src, key))
    counts = np.bincount(key, minlength=NCc * NCh)
    gstart = np.zeros(NCc * NCh + 1, np.int64)
    np.cumsum(counts, out=gstart[1:])

    # ---- occurrence-rank block filling -------------------------------
    # dma_scatter_add cannot accumulate duplicate indices within one call
    # (the CCE read-modify-write races between M2S reads and S2M writes),
    # so each block of B edges is split into rank segments: seg r holds
    # the (r+1)-th occurrences of dst values within the block, each seg
    # internally dst-unique, scattered by its own call into its own agg
    # buffer. Calls on one buffer are WAW-serialized by Tile.
    B = 2 * T

    def occ_ranks(d):
        o = np.argsort(d, kind="stable")
        sd = d[o]
        newrun = np.r_[True, sd[1:] != sd[:-1]]
        ii = np.arange(len(d))
        runstart = np.maximum.accumulate(np.where(newrun, ii, 0))
        occ = np.empty(len(d), np.int64)
        occ[o] = ii - runstart
        return occ

    prof = np.zeros(256, np.float64)
    npool = 0
    for g in range(NCc * NCh):
        dd = dst[order[gstart[g]:gstart[g + 1]]]
        for p0 in range(0, len(dd), B):
            oc = occ_ranks(dd[p0:p0 + B])
            bc = np.bincount(oc, minlength=256)[:256]
            prof += bc
            npool += 1
    prof /= max(npool, 1)
    segs = []
    for r in range(1, 256):
        if prof[r] < 24:
            break
        s_r = max(128, int(round(prof[r] / 128)) * 128)
        if sum(segs) + s_r > B - 512:
            break
        segs.append(s_r)
    SEGS = [B - sum(segs)] + segs
    cfg["SEGS"] = tuple(SEGS)
    soff = np.cumsum([0] + SEGS)

    def fill_chunk(eidx):
        blocks = []
        carry = np.empty(0, np.int64)
        ptr = 0
        n = len(eidx)
        while ptr < n or len(carry):
            take = min(B - len(carry), n - ptr)
            pool = np.concatenate([carry, eidx[ptr:ptr + take]])
            ptr += take
            oc = occ_ranks(dst[pool])
            slots = np.full(B, -1, np.int64)
            used = np.zeros(len(pool), bool)
            for r, sr in enumerate(SEGS):
                cand = np.flatnonzero(oc == r)[:sr]
                slots[soff[r]:soff[r] + len(cand)] = pool[cand]
                used[cand] = True
            carry = pool[~used]
            blocks.append(slots)
        return blocks

    core_blocks = []
    nbc = 0
    for c in range(NCc):
        per_chunk = []
        for k in range(NCh):
            g = c * NCh + k
            blks = fill_chunk(order[gstart[g]:gstart[g + 1]])
            nbc = max(nbc, len(blks))
            per_chunk.append(blks)
        core_blocks.append(per_chunk)

    tpc = 2 * nbc
    KT = tpc * T
    ETOT = NCh * KT
    cfg["TPC"], cfg["ETOT"] = tpc, ETOT

    in_maps = []
    psrc_arrs = []
    for k in range(NCh):
        tab = np.zeros((CH + 1, P), BF16)
        hi = min((k + 1) * CH, N)
        tab[: hi - k * CH] = Psrc[k * CH: hi]
        psrc_arrs.append(tab)
    gvec = np.concatenate([np.asarray(inputs["g_int"], np.float32),
                           np.asarray(inputs["g_upd"], np.float32)])[:, None]
    bvec = np.concatenate([np.asarray(inputs["be_int"], np.float32),
                           np.asarray(inputs["be_upd"], np.float32)])[:, None]
    gbn = np.asarray(inputs["g_bn"], np.float32)[:, None]
    bbn = np.asarray(inputs["be_bn"], np.float32)[:, None]

    for c in range(NCc):
        src_l = np.full(ETOT, CH, np.int16)
        dst_l = np.full(ETOT, R, np.int16)
        eft = np.zeros((FE, ETOT), BF16)
        for k in range(NCh):
            slotc = np.full(KT, -1, np.int64)
            blks = core_blocks[c][k]
            for bi, blk in enumerate(blks):
                slotc[bi * B:(bi + 1) * B] = blk
            mask = slotc >= 0
            sel = slotc[mask]
            pos = np.flatnonzero(mask) + k * KT
            src_l[pos] = (src[sel] - k * CH).astype(np.int16)
            dst_l[pos] = (dst[sel] - c * R).astype(np.int16)
            eft[:, pos] = ef[sel].T
        # verify each scatter segment is dst-unique (trash pads excluded)
        for b0 in range(0, ETOT, B):
            for r in range(len(SEGS)):
                seg = dst_l[b0 + soff[r]:b0 + soff[r + 1]]
                seg = seg[seg != R]
                assert len(np.unique(seg)) == len(seg), "seg dup!"
        pd = np.zeros((cfg["R_PAD"], P), BF16)
        pd[:R] = Pdst[c * R:(c + 1) * R]
        nft = np.zeros((F, cfg["R_PAD"]), np.float32)
        nft[:, :R] = nf[c * R:(c + 1) * R].T
        m = {
            "pdst": pd,
            "eft": eft,
            "srcidx": np.ascontiguousarray(
                np.tile(src_l.reshape(ETOT // 16, 16).T, (P // 16, 1))),
            "dstidx": np.ascontiguousarray(
                np.tile(dst_l.reshape(ETOT // 16, 16).T, (P // 16, 1))),
            "nft": nft,
            "w3": W3,
            "gvec": gvec, "bvec": bvec, "gbn": gbn, "bbn": bbn,
        }
        for k in range(NCh):
            m[f"psrc{k}"] = psrc_arrs[k]
        in_maps.append(m)
    return cfg, in_maps


def _run(inputs, T=2048, g_batch=6, trace=False):
    cfg, in_maps = _prep(inputs, T=T, g_batch=g_batch)
    ck = (cfg["N"], cfg["E"], cfg["FE"], cfg["T"], cfg["TPC"],
          cfg["G"], cfg["SEGS"])
    if ck not in _CACHE:
        _CACHE[ck] = build_graph(cfg)
    nc = _CACHE[ck]
    res = run_bass_kernel_spmd(nc, in_maps, core_ids=list(range(cfg["NC"])),
                               trace=trace)
    R = cfg["R"]
    out = np.concatenate(
        [np.asarray(res.results[c]["outT"])[:, :R].T for c in range(cfg["NC"])],
        axis=0)
    return np.ascontiguousarray(out, dtype=np.float32), res


def kernel(**inputs) -> np.ndarray:
    out, _ = _run(inputs)
    return out



# revision 2
# speedup vs baseline: 1.6942x; 1.6942x over previous
"""CGCNN conv kernel for 8 TRN2 NeuronCores (Bass/Tile).

Strategy (edge-parallel, dst-sharded, row-major):
  z @ W = nf[src] @ W[0:64] + nf[dst] @ W[64:128] + ef @ W[128:160]
  - Host precomputes P_src = nf @ [Wi[:64]|Wu[:64]]  (bf16 [N,128], 256B rows)
                    P_dst = nf @ [Wi[64:128]|Wu[64:128]]
  - Edges sorted by (dst//R, src//CH, src); core c owns dst range
    [c*R,(c+1)*R) so the segment-sum is core-local; src chunks of CH=25000
    keep gather indices in int16.
  - Pass 1 (row-major): non-transposed dma_gather of P rows (512-idx calls
    rotated over 4 SWDGE queues); PE computes ef@W3 into row-major PSUM;
    DVE adds the gathered rows; per-feature BN stats via strided reduces
    accumulated in [128,128] partials; x stored to DRAM bf16 row-major.
  - AllReduce [1,256] edge-BN stats; scale/shift broadcast to [128,128]
    tiles via PE ones-outer-product.
  - Pass 2: reload x; BN applied with broadcast_to DVE ops; ACT sigmoid /
    exp / ln(1+u); msg = gate*sp row-major feeds dma_scatter_add directly
    (occurrence-rank segments into rotating agg buffers, queue-rotated).
  - Phase 3: node-BN stats AllReduce [F,2]; out = softplus(nf + bn(agg))
    feat-major; host transposes back.
"""

import itertools
import math
import sys

import numpy as np

for _p in ("/opt/trn_rl_repo", "/root/.axon_site/_ro/trn_rl_repo"):
    if _p not in sys.path:
        sys.path.append(_p)

import ml_dtypes
from concourse import bacc, bass, mybir
from concourse import tile as ctile
from concourse.bass_utils import run_bass_kernel_spmd
from concourse.masks import make_identity

P = 128
F = 64  # node feature dim; 2F == P
EPS = 1e-5
BF16 = ml_dtypes.bfloat16
NQ = 4  # SWDGE queues
GQ = 512  # indices per gather call

Alu = mybir.AluOpType
Act = mybir.ActivationFunctionType
dt = mybir.dt


def _cfg(N, E, FE, T=2048, ncores=8):
    R = N // ncores
    assert R * ncores == N
    nchunk = max(1, math.ceil(N / 25000))
    CH = math.ceil(N / nchunk)
    assert CH + 1 <= 32768 and R + 1 <= 32768
    r_pad = math.ceil((R + 1) / P) * P
    return dict(
        N=N, E=E, FE=FE, T=T, NC=ncores,
        R=R, NCHUNK=nchunk, CH=CH, R_PAD=r_pad,
    )


def build_graph(cfg):
    NC, T, FE = cfg["NC"], cfg["T"], cfg["FE"]
    CH, NCHUNK, R_PAD = cfg["CH"], cfg["NCHUNK"], cfg["R_PAD"]
    TPC, ETOT = cfg["TPC"], cfg["ETOT"]
    SEGS = list(cfg["SEGS"])
    nseg = len(SEGS)
    NTILES = NCHUNK * TPC
    NBLK = NTILES // 2
    NGRP = R_PAD // P
    G = T // P  # row-major groups per tile
    B = 2 * T
    inv_e = 1.0 / float(cfg["E"])
    inv_n = 1.0 / float(cfg["N"])
    qc = itertools.count()

    nc = bacc.Bacc("TRN2", target_bir_lowering=False, debug=False,
                   num_devices=NC, num_swdge_queues=NQ)

    psrc = [nc.dram_tensor(f"psrc{c}", [CH + 1, P], dt.bfloat16,
                           kind="ExternalInput") for c in range(NCHUNK)]
    pdst = nc.dram_tensor("pdst", [R_PAD, P], dt.bfloat16, kind="ExternalInput")
    eft = nc.dram_tensor("eft", [FE, ETOT], dt.bfloat16, kind="ExternalInput")
    srcidx = nc.dram_tensor("srcidx", [P, ETOT // 16], dt.int16,
                            kind="ExternalInput")
    dstidx = nc.dram_tensor("dstidx", [P, ETOT // 16], dt.int16,
                            kind="ExternalInput")
    nft = nc.dram_tensor("nft", [F, R_PAD], dt.float32, kind="ExternalInput")
    w3 = nc.dram_tensor("w3", [FE, P], dt.bfloat16, kind="ExternalInput")
    gvr = nc.dram_tensor("gvr", [1, P], dt.float32, kind="ExternalInput")
    bvr = nc.dram_tensor("bvr", [1, P], dt.float32, kind="ExternalInput")
    gbn = nc.dram_tensor("gbn", [F, 1], dt.float32, kind="ExternalInput")
    bbn = nc.dram_tensor("bbn", [F, 1], dt.float32, kind="ExternalInput")
    outT = nc.dram_tensor("outT", [F, R_PAD], dt.float32, kind="ExternalOutput")

    xrow = nc.dram_tensor("xrow", [NBLK, P, B], dt.bfloat16, kind="Internal")
    aggd = [nc.dram_tensor(f"aggd{r}", [NGRP, P, F], dt.float32,
                           kind="Internal") for r in range(nseg + 1)]
    cc1i = nc.dram_tensor("cc1i", [1, 2 * P], dt.float32, kind="Internal")
    cc1o = nc.dram_tensor("cc1o", [1, 2 * P], dt.float32, kind="Internal",
                          addr_space="Shared")
    cc2i = nc.dram_tensor("cc2i", [F, 2], dt.float32, kind="Internal")
    cc2o = nc.dram_tensor("cc2o", [F, 2], dt.float32, kind="Internal",
                          addr_space="Shared")
    rg = [list(range(NC))]

    with ctile.TileContext(nc) as tc:
        with tc.tile_pool(name="const", bufs=1) as cp:
            w3_sb = cp.tile([FE, P], dt.bfloat16)
            nc.sync.dma_start(w3_sb[:], w3.ap())
            identf128 = cp.tile([P, P], dt.float32)
            make_identity(nc, identf128[:])
            gv = cp.tile([1, P], dt.float32)
            nc.sync.dma_start(gv[:], gvr.ap())
            bv = cp.tile([1, P], dt.float32)
            nc.sync.dma_start(bv[:], bvr.ap())
            gbn_sb = cp.tile([F, 1], dt.float32)
            nc.sync.dma_start(gbn_sb[:], gbn.ap())
            bbn_sb = cp.tile([F, 1], dt.float32)
            nc.sync.dma_start(bbn_sb[:], bbn.ap())
            ones1 = cp.tile([1, P], dt.float32)
            nc.vector.memset(ones1[:], 1.0)
            onesc = cp.tile([P, 1], dt.float32)
            nc.vector.memset(onesc[:], 1.0)

            accs = cp.tile([P, P], dt.float32)
            nc.vector.memset(accs[:], 0.0)
            accq = cp.tile([P, P], dt.float32)
            nc.vector.memset(accq[:], 0.0)
            svbc = cp.tile([P, P], dt.float32)
            bvbc = cp.tile([P, P], dt.float32)

            # zero-fill agg accumulators
            zb = cp.tile([P, 512], dt.float32)
            nc.vector.memset(zb[:], 0.0)
            gper = 512 // F
            for r in range(nseg + 1):
                for g0 in range(0, NGRP, gper):
                    ng = min(gper, NGRP - g0)
                    nc.sync.dma_start(aggd[r].ap()[g0:g0 + ng, :, :],
                                      zb[:, :ng * F])

            # ---------------- pass 1 (row-major) ----------------
            with tc.tile_pool(name="p1", bufs=3) as p1, \
                 tc.tile_pool(name="p1i", bufs=4) as p1i, \
                 tc.tile_pool(name="ps1", bufs=4, space="PSUM") as ps1:
                for c in range(NCHUNK):
                    for tl in range(TPC):
                        t = c * TPC + tl
                        sidx = p1i.tile([P, T // 16], dt.int16, tag="sidx")
                        nc.sync.dma_start(
                            sidx[:], srcidx.ap()[:, t * (T // 16):
                                                 (t + 1) * (T // 16)])
                        didx = p1i.tile([P, T // 16], dt.int16, tag="didx")
                        nc.sync.dma_start(
                            didx[:], dstidx.ap()[:, t * (T // 16):
                                                 (t + 1) * (T // 16)])
                        srcg = p1.tile([P, G, P], dt.bfloat16, tag="srcg")
                        dstg = p1.tile([P, G, P], dt.bfloat16, tag="dstg")
                        for q in range(T // GQ):
                            gs = slice(q * (GQ // P), (q + 1) * (GQ // P))
                            qi = slice(q * (GQ // 16), (q + 1) * (GQ // 16))
                            nc.gpsimd.dma_gather(
                                srcg[:, gs, :], psrc[c].ap(), sidx[:, qi],
                                GQ, GQ, P, queue_num=next(qc) % NQ)
                            nc.gpsimd.dma_gather(
                                dstg[:, gs, :], pdst.ap(), didx[:, qi],
                                GQ, GQ, P, queue_num=next(qc) % NQ)
                        eftt = p1.tile([FE, T], dt.bfloat16, tag="eftt")
                        nc.sync.dma_start(eftt[:], eft.ap()[:, t * T:(t + 1) * T])

                        xh = p1.tile([P, T], dt.bfloat16, tag="xh")
                        for s in range(T // 512):
                            ps = ps1.tile([P, 512], dt.float32, tag="ps")
                            for g in range(4):
                                col = (s * 4 + g) * P
                                nc.tensor.matmul(
                                    ps[:, g * P:(g + 1) * P],
                                    eftt[:, col:col + P], w3_sb[:],
                                    start=True, stop=True)
                            sl = slice(s * 512, (s + 1) * 512)
                            srcf = srcg[:].rearrange("p a b -> p (a b)")
                            dstf = dstg[:].rearrange("p a b -> p (a b)")
                            nc.vector.tensor_tensor(
                                xh[:, sl], ps[:], srcf[:, sl], Alu.add)
                            nc.vector.tensor_tensor(
                                xh[:, sl], xh[:, sl], dstf[:, sl], Alu.add)
                        xsq = p1.tile([P, T], dt.bfloat16, tag="xsq")
                        nc.scalar.activation(xsq[:], xh[:], Act.Square)
                        rs = p1.tile([P, P], dt.float32, tag="rs")
                        nc.vector.tensor_reduce(
                            rs[:], xh[:].rearrange("p (g f) -> p f g", f=P),
                            mybir.AxisListType.X, Alu.add)
                        nc.vector.tensor_tensor(accs[:], accs[:], rs[:],
                                                Alu.add)
                        rq = p1.tile([P, P], dt.float32, tag="rq")
                        nc.vector.tensor_reduce(
                            rq[:], xsq[:].rearrange("p (g f) -> p f g", f=P),
                            mybir.AxisListType.X, Alu.add)
                        nc.vector.tensor_tensor(accq[:], accq[:], rq[:],
                                                Alu.add)
                        nc.scalar.dma_start(
                            xrow.ap()[t // 2, :, (t % 2) * T:(t % 2 + 1) * T],
                            xh[:])

            # ---------------- edge-BN stats ----------------
            with tc.tile_pool(name="st", bufs=1) as stp, \
                 tc.tile_pool(name="pst", bufs=2, space="PSUM") as pst:
                sps = pst.tile([1, P], dt.float32, tag="sps")
                nc.tensor.matmul(sps[:], onesc[:], accs[:],
                                 start=True, stop=True)
                qps = pst.tile([1, P], dt.float32, tag="qps")
                nc.tensor.matmul(qps[:], onesc[:], accq[:],
                                 start=True, stop=True)
                cst = stp.tile([1, 2 * P], dt.float32)
                nc.vector.tensor_copy(cst[:, 0:P], sps[:])
                nc.vector.tensor_copy(cst[:, P:2 * P], qps[:])
                nc.sync.dma_start(cc1i.ap(), cst[:])
                nc.gpsimd.collective_compute(
                    "AllReduce", Alu.add, replica_groups=rg,
                    ins=[cc1i.ap().opt()], outs=[cc1o.ap().opt()])
                gst = stp.tile([1, 2 * P], dt.float32)
                nc.sync.dma_start(gst[:], cc1o.ap())

                mu = stp.tile([1, P], dt.float32)
                nc.vector.tensor_scalar(mu[:], gst[:, 0:P], inv_e, None,
                                        Alu.mult)
                veps = stp.tile([1, P], dt.float32)
                musq = stp.tile([1, P], dt.float32)
                nc.vector.tensor_tensor(musq[:], mu[:], mu[:], Alu.mult)
                nc.vector.tensor_scalar(veps[:], gst[:, P:2 * P], inv_e, None,
                                        Alu.mult)
                nc.vector.tensor_tensor(veps[:], veps[:], musq[:],
                                        Alu.subtract)
                nc.vector.tensor_scalar(veps[:], veps[:], EPS, None, Alu.add)
                sdv = stp.tile([1, P], dt.float32)
                nc.scalar.sqrt(sdv[:], veps[:])
                isd = stp.tile([1, P], dt.float32)
                nc.vector.reciprocal(isd[:], sdv[:])
                scl = stp.tile([1, P], dt.float32)
                nc.vector.tensor_tensor(scl[:], gv[:], isd[:], Alu.mult)
                shf = stp.tile([1, P], dt.float32)
                nc.vector.tensor_tensor(shf[:], mu[:], scl[:], Alu.mult)
                nc.vector.tensor_tensor(shf[:], bv[:], shf[:], Alu.subtract)

                bps = pst.tile([P, P], dt.float32, tag="bps")
                nc.tensor.matmul(bps[:], ones1[:], scl[:], start=True,
                                 stop=True)
                nc.vector.tensor_copy(svbc[:], bps[:])
                bps2 = pst.tile([P, P], dt.float32, tag="bps")
                nc.tensor.matmul(bps2[:], ones1[:], shf[:], start=True,
                                 stop=True)
                nc.vector.tensor_copy(bvbc[:], bps2[:])

            # ---------------- pass 2 ----------------
            GB = B // P  # 32 row-groups per block
            soff = np.cumsum([0] + SEGS)
            with tc.tile_pool(name="p2", bufs=3) as p2, \
                 tc.tile_pool(name="p2i", bufs=3) as p2i:
                for b in range(NBLK):
                    xi = p2.tile([P, GB, P], dt.bfloat16, tag="xi")
                    nc.sync.dma_start(
                        xi[:], xrow.ap()[b].rearrange("p (a b) -> p a b", b=P))
                    xn = p2.tile([P, GB, P], dt.float32, tag="xn")
                    nc.vector.tensor_tensor(
                        xn[:], xi[:],
                        svbc[:, None, :].broadcast_to([P, GB, P]), Alu.mult)
                    nc.vector.tensor_tensor(
                        xn[:], xn[:],
                        bvbc[:, None, :].broadcast_to([P, GB, P]), Alu.add)
                    gate = p2.tile([P, GB, F], dt.bfloat16, tag="gate")
                    nc.scalar.activation(gate[:], xn[:, :, 0:F], Act.Sigmoid)
                    u = p2.tile([P, GB, F], dt.float32, tag="u")
                    nc.scalar.activation(u[:], xn[:, :, F:P], Act.Exp)
                    sp = p2.tile([P, GB, F], dt.float32, tag="sp")
                    nc.scalar.activation(sp[:], u[:], Act.Ln, bias=1.0,
                                         scale=1.0)
                    msg = p2.tile([P, GB, F], dt.float32, tag="msg")
                    nc.vector.tensor_tensor(msg[:], gate[:], sp[:], Alu.mult)
                    didx2 = p2i.tile([P, B // 16], dt.int16, tag="didx2")
                    nc.sync.dma_start(
                        didx2[:],
                        dstidx.ap()[:, b * (B // 16):(b + 1) * (B // 16)])
                    for r, sr in enumerate(SEGS):
                        ri = (nseg if (r == 0 and b % 2) else r)
                        o0 = int(soff[r])
                        nc.gpsimd.dma_scatter_add(
                            aggd[ri].ap().flatten_outer_dims(),
                            msg[:, o0 // P:(o0 + sr) // P, :],
                            didx2[:, o0 // 16:(o0 + sr) // 16],
                            sr, sr, F, queue_num=next(qc) % NQ)

            # ---------------- phase 3 (chunked over node groups) ------
            with tc.tile_pool(name="p3", bufs=1) as p3, \
                 tc.tile_pool(name="p3c", bufs=3) as p3c, \
                 tc.tile_pool(name="p3w", bufs=2) as p3w, \
                 tc.tile_pool(name="ps3", bufs=4, space="PSUM") as ps3:
                gpt = 512 // P  # groups per psum tile / chunk
                aggT = p3.tile([F, NGRP * P], dt.float32)
                for q0 in range(0, NGRP, gpt):
                    nq_ = min(gpt, NGRP - q0)
                    ac = p3c.tile([P, gpt, F], dt.float32, tag="ac")
                    nc.sync.dma_start(
                        ac[:, :nq_, :],
                        aggd[0].ap()[q0:q0 + nq_].rearrange("g p d -> p g d"))
                    for r in range(1, nseg + 1):
                        at = p3c.tile([P, gpt, F], dt.float32, tag="at")
                        nc.sync.dma_start(
                            at[:, :nq_, :],
                            aggd[r].ap()[q0:q0 + nq_].rearrange(
                                "g p d -> p g d"))
                        nc.vector.tensor_tensor(ac[:, :nq_, :], ac[:, :nq_, :],
                                                at[:, :nq_, :], Alu.add)
                    pstt = ps3.tile([F, 512], dt.float32, tag="pst3")
                    for k in range(nq_):
                        nc.tensor.transpose(
                            pstt[:, k * P:(k + 1) * P],
                            ac[:, k, :], identf128[:])
                    nc.vector.tensor_copy(
                        aggT[:, q0 * P:(q0 + nq_) * P], pstt[:, :nq_ * P])

                Rr = cfg["R"]
                if R_PAD > Rr:
                    nc.vector.memset(aggT[:, Rr:], 0.0)
                nchunk3 = 8
                cb = [(Rr * i) // nchunk3 for i in range(nchunk3 + 1)]
                nsum = p3.tile([F, 2 * nchunk3], dt.float32)
                for i in range(nchunk3):
                    sl = slice(cb[i], cb[i + 1])
                    nc.vector.tensor_reduce(nsum[:, 2 * i:2 * i + 1],
                                            aggT[:, sl],
                                            mybir.AxisListType.X, Alu.add)
                    sq = p3w.tile([F, (NGRP * P) // nchunk3 + P], dt.float32,
                                  tag="sq")
                    w = cb[i + 1] - cb[i]
                    nc.vector.tensor_tensor(sq[:, :w], aggT[:, sl],
                                            aggT[:, sl], Alu.mult)
                    nc.vector.tensor_reduce(nsum[:, 2 * i + 1:2 * i + 2],
                                            sq[:, :w],
                                            mybir.AxisListType.X, Alu.add)
                nsum2 = p3.tile([F, 2], dt.float32)
                nc.vector.tensor_reduce(
                    nsum2[:, 0:1],
                    nsum[:].rearrange("p (a b) -> p b a", b=2)[:, 0, :],
                    mybir.AxisListType.X, Alu.add)
                nc.vector.tensor_reduce(
                    nsum2[:, 1:2],
                    nsum[:].rearrange("p (a b) -> p b a", b=2)[:, 1, :],
                    mybir.AxisListType.X, Alu.add)
                nsum = nsum2
                nc.sync.dma_start(cc2i.ap(), nsum[:])
                nc.gpsimd.collective_compute(
                    "AllReduce", Alu.add, replica_groups=rg,
                    ins=[cc2i.ap().opt()], outs=[cc2o.ap().opt()])
                gs2 = p3.tile([F, 2], dt.float32)
                nc.sync.dma_start(gs2[:], cc2o.ap())

                mu2 = p3.tile([F, 1], dt.float32)
                nc.vector.tensor_scalar(mu2[:], gs2[:, 0:1], inv_n, None,
                                        Alu.mult)
                ve2 = p3.tile([F, 1], dt.float32)
                ms2 = p3.tile([F, 1], dt.float32)
                nc.vector.tensor_tensor(ms2[:], mu2[:], mu2[:], Alu.mult)
                nc.vector.tensor_scalar(ve2[:], gs2[:, 1:2], inv_n, None,
                                        Alu.mult)
                nc.vector.tensor_tensor(ve2[:], ve2[:], ms2[:], Alu.subtract)
                nc.vector.tensor_scalar(ve2[:], ve2[:], EPS, None, Alu.add)
                sd2 = p3.tile([F, 1], dt.float32)
                nc.scalar.sqrt(sd2[:], ve2[:])
                is2 = p3.tile([F, 1], dt.float32)
                nc.vector.reciprocal(is2[:], sd2[:])
                sc2 = p3.tile([F, 1], dt.float32)
                nc.vector.tensor_tensor(sc2[:], gbn_sb[:], is2[:], Alu.mult)
                sh2 = p3.tile([F, 1], dt.float32)
                nc.vector.tensor_tensor(sh2[:], mu2[:], sc2[:], Alu.mult)
                nc.vector.tensor_tensor(sh2[:], bbn_sb[:], sh2[:],
                                        Alu.subtract)

                cw = ((NGRP // nchunk3) + 1) * P
                for i in range(nchunk3):
                    c0 = min(NGRP * P, i * cw)
                    c1 = min(NGRP * P, (i + 1) * cw)
                    if c1 <= c0:
                        continue
                    w = c1 - c0
                    nftc = p3w.tile([F, cw], dt.float32, tag="nftc")
                    nc.sync.dma_start(nftc[:, :w], nft.ap()[:, c0:c1])
                    s1 = p3w.tile([F, cw], dt.float32, tag="s1")
                    nc.vector.tensor_scalar(s1[:, :w], aggT[:, c0:c1],
                                            sc2[:], sh2[:], Alu.mult, Alu.add)
                    nc.vector.tensor_tensor(s1[:, :w], s1[:, :w], nftc[:, :w],
                                            Alu.add)
                    u3 = p3w.tile([F, cw], dt.float32, tag="u3")
                    nc.scalar.activation(u3[:, :w], s1[:, :w], Act.Exp)
                    o3 = p3w.tile([F, cw], dt.float32, tag="o3")
                    nc.scalar.activation(o3[:, :w], u3[:, :w], Act.Ln,
                                         bias=1.0, scale=1.0)
                    nc.sync.dma_start(outT.ap()[:, c0:c1], o3[:, :w])

    nc.compile()
    return nc


_CACHE = {}


def _prep(inputs, T=2048):
    nf = np.ascontiguousarray(np.asarray(inputs["node_feats"], np.float32))
    ef = np.ascontiguousarray(np.asarray(inputs["edge_feats"], np.float32))
    src = np.asarray(inputs["src"], np.int64)
    dst = np.asarray(inputs["dst"], np.int64)
    Wi = np.asarray(inputs["W_int"], np.float32)
    Wu = np.asarray(inputs["W_upd"], np.float32)
    N, Fn = nf.shape
    E, FE = ef.shape
    assert Fn == F
    cfg = _cfg(N, E, FE, T=T)
    NCh, CH, R, NCc = cfg["NCHUNK"], cfg["CH"], cfg["R"], cfg["NC"]

    # b_int/b_upd are dropped: a constant bias shifts mean equally and
    # cancels inside BatchNorm.
    Psrc = (nf @ np.concatenate([Wi[:F], Wu[:F]], axis=1)).astype(BF16)
    Pdst = (nf @ np.concatenate([Wi[F:2 * F], Wu[F:2 * F]], axis=1)).astype(BF16)
    W3 = np.concatenate([Wi[2 * F:], Wu[2 * F:]], axis=1).astype(BF16)

    core = dst // R
    chunk = src // CH
    key = core * NCh + chunk
    order = np.lexsort((src, key))
    counts = np.bincount(key, minlength=NCc * NCh)
    gstart = np.zeros(NCc * NCh + 1, np.int64)
    np.cumsum(counts, out=gstart[1:])

    # ---- occurrence-rank block filling -------------------------------
    # dma_scatter_add cannot accumulate duplicate indices within one call
    # (the CCE read-modify-write races between M2S reads and S2M writes),
    # so each block of B edges is split into rank segments: seg r holds
    # the (r+1)-th occurrences of dst values within the block, each seg
    # internally dst-unique, scattered by its own call into its own agg
    # buffer. Calls on one buffer are WAW-serialized by Tile.
    B = 2 * T

    def occ_ranks(d):
        o = np.argsort(d, kind="stable")
        sd = d[o]
        newrun = np.r_[True, sd[1:] != sd[:-1]]
        ii = np.arange(len(d))
        runstart = np.maximum.accumulate(np.where(newrun, ii, 0))
        occ = np.empty(len(d), np.int64)
        occ[o] = ii - runstart
        return occ

    prof = np.zeros(256, np.float64)
    npool = 0
    for g in range(NCc * NCh):
        dd = dst[order[gstart[g]:gstart[g + 1]]]
        for p0 in range(0, len(dd), B):
            oc = occ_ranks(dd[p0:p0 + B])
            bc = np.bincount(oc, minlength=256)[:256]
            prof += bc
            npool += 1
    prof /= max(npool, 1)
    segs = []
    for r in range(1, 256):
        if prof[r] < 24:
            break
        s_r = max(128, int(round(prof[r] / 128)) * 128)
        if sum(segs) + s_r > B - 512:
            break
        segs.append(s_r)
    SEGS = [B - sum(segs)] + segs
    cfg["SEGS"] = tuple(SEGS)
    soff = np.cumsum([0] + SEGS)

    def fill_chunk(eidx):
        blocks = []
        carry = np.empty(0, np.int64)
        ptr = 0
        n = len(eidx)
        while ptr < n or len(carry):
            take = min(B - len(carry), n - ptr)
            pool = np.concatenate([carry, eidx[ptr:ptr + take]])
            ptr += take
            oc = occ_ranks(dst[pool])
            slots = np.full(B, -1, np.int64)
            used = np.zeros(len(pool), bool)
            for r, sr in enumerate(SEGS):
                cand = np.flatnonzero(oc == r)[:sr]
                slots[soff[r]:soff[r] + len(cand)] = pool[cand]
                used[cand] = True
            carry = pool[~used]
            blocks.append(slots)
        return blocks

    core_blocks = []
    nbc = 0
    for c in range(NCc):
        per_chunk = []
        for k in range(NCh):
            g = c * NCh + k
            blks = fill_chunk(order[gstart[g]:gstart[g + 1]])
            nbc = max(nbc, len(blks))
            per_chunk.append(blks)
        core_blocks.append(per_chunk)

    tpc = 2 * nbc
    KT = tpc * T
    ETOT = NCh * KT
    cfg["TPC"], cfg["ETOT"] = tpc, ETOT

    in_maps = []
    psrc_arrs = []
    for k in range(NCh):
        tab = np.zeros((CH + 1, P), BF16)
        hi = min((k + 1) * CH, N)
        tab[: hi - k * CH] = Psrc[k * CH: hi]
        psrc_arrs.append(tab)
    gvec = np.concatenate([np.asarray(inputs["g_int"], np.float32),
                           np.asarray(inputs["g_upd"], np.float32)])[None, :]
    bvec = np.concatenate([np.asarray(inputs["be_int"], np.float32),
                           np.asarray(inputs["be_upd"], np.float32)])[None, :]
    gbn = np.asarray(inputs["g_bn"], np.float32)[:, None]
    bbn = np.asarray(inputs["be_bn"], np.float32)[:, None]

    for c in range(NCc):
        src_l = np.full(ETOT, CH, np.int16)
        dst_l = np.full(ETOT, R, np.int16)
        eftc = np.zeros((FE, ETOT), BF16)
        for k in range(NCh):
            slotc = np.full(KT, -1, np.int64)
            blks = core_blocks[c][k]
            for bi, blk in enumerate(blks):
                slotc[bi * B:(bi + 1) * B] = blk
            mask = slotc >= 0
            sel = slotc[mask]
            pos = np.flatnonzero(mask) + k * KT
            src_l[pos] = (src[sel] - k * CH).astype(np.int16)
            dst_l[pos] = (dst[sel] - c * R).astype(np.int16)
            eftc[:, pos] = ef[sel].T
        pd = np.zeros((cfg["R_PAD"], P), BF16)
        pd[:R] = Pdst[c * R:(c + 1) * R]
        nftc = np.zeros((F, cfg["R_PAD"]), np.float32)
        nftc[:, :R] = nf[c * R:(c + 1) * R].T
        m = {
            "pdst": pd,
            "eft": eftc,
            "srcidx": np.ascontiguousarray(
                np.tile(src_l.reshape(ETOT // 16, 16).T, (P // 16, 1))),
            "dstidx": np.ascontiguousarray(
                np.tile(dst_l.reshape(ETOT // 16, 16).T, (P // 16, 1))),
            "nft": nftc,
            "w3": W3,
            "gvr": gvec, "bvr": bvec, "gbn": gbn, "bbn": bbn,
        }
        for k in range(NCh):
            m[f"psrc{k}"] = psrc_arrs[k]
        in_maps.append(m)
    return cfg, in_maps


def _run(inputs, T=2048, trace=False):
    cfg, in_maps = _prep(inputs, T=T)
    ck = (cfg["N"], cfg["E"], cfg["FE"], cfg["T"], cfg["TPC"], cfg["SEGS"])
    if ck not in _CACHE:
        _CACHE[ck] = build_graph(cfg)
    nc = _CACHE[ck]
    res = run_bass_kernel_spmd(nc, in_maps, core_ids=list(range(cfg["NC"])),
                               trace=trace)
    R = cfg["R"]
    out = np.concatenate(
        [np.asarray(res.results[c]["outT"])[:, :R].T for c in range(cfg["NC"])],
        axis=0)
    return np.ascontiguousarray(out, dtype=np.float32), res


def kernel(**inputs) -> np.ndarray:
    out, _ = _run(inputs)
    return out
